# revision 37
# baseline (speedup 1.0000x reference)
"""Trainium2 Bass kernel for a GQA attention block (NeuronAttentionBase).

Shapes: B=1, S=2048, H=4096, NH=32 query heads, NKV=8 kv heads, D=128.
Sharding: tensor-parallel across heads on 8 NeuronCores — 4 query heads +
1 kv head per core; Wq/Wk/Wv column-sharded.

Out-projection is sharded 2D over (seq-group, feature-half): core c
computes Y^T[fh*2048:(fh+1)*2048, sg*512:(sg+1)*512] with sg=c>>1,
fh=c&1. Feeding it needs each core's per-head attention output only for
ONE 512-seq group on each consumer, so a single 4MB AllToAll replaces
the previous 2x8MB AllGather (14MB received per core -> 3.7MB): slot d
of the A2A input carries this core's 4 heads for seq group d>>1. Wo is
streamed from HBM during phase 3 (16.8MB over ~110us of PE work)
instead of being SBUF-resident.

All compute runs in "transposed space" (activations stored as [feature,
seq] tiles) so no on-device transposes are needed anywhere:
  Q^T/K^T  = matmul(lhsT=W, rhs=X^T)        -> [d, s]
  V        = matmul(lhsT=X^T_blk, rhs=Wv)    -> [s, d]   (natural)
  S^T      = matmul(lhsT=K^T_blk, rhs=Q^T)   -> [k, q]
  P~^T     = exp(S^T/sqrt(D)) * causal_mask  (no max subtraction; scores
             are O(10) for this distribution so fp32 exp is safe)
  OUT^T    = matmul(lhsT=V_blk, rhs=P~^T)    -> [d, q]  (+ rowsums;
             normalization applied on PSUM eviction)
  Y^T_blk  = matmul(lhsT=Wo_ft, rhs=alltoall(OUT^T)) -> [128, 512]

Phase 2 is software-pipelined: the scores matmuls for block-group g+1
are issued before the exp/PV of group g, so the in-order PE queue never
stalls on the ACT engine's exp latency.
"""

import math
import os

import numpy as np
import ml_dtypes


def _env(name, default):
    v = os.environ.get(f"K_{name}")
    if v is None:
        return default
    if isinstance(default, bool):
        return v not in ("0", "false", "False")
    if isinstance(default, int):
        return int(v)
    return v

import concourse.bass as bass
import concourse.mybir as mybir
import concourse.tile as tile
from concourse import bacc
from concourse.masks import make_identity

N_CORES = 8
S = 2048
H = 4096
NH, NKV, D = 32, 8, 128
HPC = NH // N_CORES          # query heads per core = 4
QO = HPC * D                 # per-core Wq out cols = 512
FH = H // 2                  # feature half = 2048 (phase3 cols per core)
FT = FH // 128               # 16 feature tiles per core
HC = H // 128                # 32 contraction chunks
SC = S // 512                # 4 seq chunks of 512
SB = S // 128                # 16 seq blocks of 128
ROPE_THETA = 10000.0
MASK_ENGINE = _env("MASK_ENGINE", "vector")   # "vector" | "gpsimd"
ROWSUM = _env("ROWSUM", "pe")  # "pe" ones-matmul in PSUM (no cross-engine
                             # stall) | "mix" 2:1 dve/gps | "gps" | "dve"
SPS = _env("SPS", 512)       # scores tile width: 512 (lookahead 2) | 1024
ROPE_SPLIT = _env("ROPE_SPLIT", False)  # rope eviction across DVE + gpsimd
FIN_DEFER = _env("FIN_DEFER", False)
Y_DIRECT = _env("Y_DIRECT", False)
XG_N = _env("XG_N", 8)       # hc chunks per phase1 DMA
XTP_BUFS = _env("XTP_BUFS", 3)   # phase1 xt prefetch depth
LA_DEEP = _env("LA_DEEP", False)
P_SLIM = _env("P_SLIM", False)
DMA_ALT = _env("DMA_ALT", False)
TR_BUFS = _env("TR_BUFS", 2)
PE_LA3 = _env("PE_LA3", False)
ROPE_ACT = _env("ROPE_ACT", True)
RS_BCAST = _env("RS_BCAST", False)
WOP_BUFS = _env("WOP_BUFS", 3)   # phase3 Wo stream prefetch depth
FUSE12 = _env("FUSE12", False)   # interleave proj/attn per seq chunk
PS1_HOIST = _env("PS1_HOIST", True)  # one ps1 PSUM pool for all of phase1
PIPE = _env("PIPE", True)   # overlap rep r's A2A+phase3 with rep r+1

bf = mybir.dt.bfloat16
f32 = mybir.dt.float32
AF = mybir.ActivationFunctionType


def build_nc(timing=False, phases=(1, 2, 3), single=False,
             phases_only=False, reps=1, skip_cc=False, reps_phases=None):
    """timing=R (int>0) wraps the compute phases (not the collectives)
    in a hardware For_i loop, so device time per iteration can be
    measured as (wall(R) - wall(1)) / (R-1).

    reps=R (static unroll, timing=False path only) repeats the FULL
    kernel — compute phases AND the AllToAll collective — R times in
    one NEFF. (wall(reps=R) - wall(reps=1)) / (R-1) is the complete
    per-kernel device time including collectives."""
    nc = bacc.Bacc(None, target_bir_lowering=False, debug=False,
                   num_devices=1 if single else N_CORES)
    # xt laid out seq-chunk-major so each phase-1 fetch is contiguous
    # per partition; wo laid out feature-tile-major likewise
    xt = nc.dram_tensor("xt", [128, SC, HC, 512], bf, kind="ExternalInput")
    wq = nc.dram_tensor("wq", [128, HC, QO], bf, kind="ExternalInput")
    wk = nc.dram_tensor("wk", [128, HC, D], bf, kind="ExternalInput")
    wv = nc.dram_tensor("wv", [128, HC, D], bf, kind="ExternalInput")
    wo = nc.dram_tensor("wo", [128, FT, NH, 128], bf, kind="ExternalInput")
    fsin = nc.dram_tensor("fsin", [128, S], f32, kind="ExternalInput")
    fcos = nc.dram_tensor("fcos", [128, S], f32, kind="ExternalInput")
    msk = nc.dram_tensor("msk", [128, 1024], bf, kind="ExternalInput")
    y = nc.dram_tensor("y", [FH, 512], f32, kind="ExternalOutput")

    scale = 1.0 / math.sqrt(D)

    with tile.TileContext(nc) as tc:
        with (
            tc.tile_pool(name="wts", bufs=1) as wts,
            tc.tile_pool(name="pers", bufs=1) as pers,
            tc.tile_pool(name="xtp", bufs=3) as xtp,
            tc.tile_pool(name="work", bufs=3) as work,
            tc.tile_pool(name="ppool", bufs=3) as ppool,
            tc.tile_pool(name="gpool", bufs=2) as gpool,
            tc.tile_pool(name="dram", bufs=1, space="DRAM") as dram,
        ):
            # ---- resident weights ----
            wq_sb = wts.tile([128, HC, QO], bf, tag="wq")
            wk_sb = wts.tile([128, HC, D], bf, tag="wk")
            wv_sb = wts.tile([128, HC, D], bf, tag="wv")
            nc.sync.dma_start(wq_sb[:], wq[:])
            nc.sync.dma_start(wk_sb[:], wk[:])
            nc.sync.dma_start(wv_sb[:], wv[:])

            msk_sb = wts.tile([128, 1024], bf, tag="msk")
            nc.sync.dma_start(msk_sb[:], msk[:])

            # ---- RoPE cos/sin tables (args pre-reduced to [-pi, pi)),
            # computed in 512-col chunks through the shared work tiles ----
            cos_sb = pers.tile([128, S], f32, tag="cos")
            sin_sb = pers.tile([128, S], f32, tag="sin")
            for sc_i in range(SC):
                sl = bass.ts(sc_i, 512)
                ftmp = work.tile([128, 512], f32, tag="rot", bufs=2,
                                 name="ftmp")
                nc.sync.dma_start(ftmp[:], fsin[:, sl])
                nc.scalar.activation(sin_sb[:, sl], ftmp[:], AF.Sin)
                ftmp2 = work.tile([128, 512], f32, tag="t1", bufs=2,
                                  name="ftmp2")
                nc.sync.dma_start(ftmp2[:], fcos[:, sl])
                nc.scalar.activation(cos_sb[:, sl], ftmp2[:], AF.Sin)

            # ---- constants ----
            ones128 = wts.tile([128, 128], f32, tag="ones128")
            nc.any.memset(ones128[:], 1.0)
            ones_col = wts.tile([128, 1], bf, tag="ones_col")
            nc.any.memset(ones_col[:], 1.0)
            ones_1x = wts.tile([1, 128], f32, tag="ones_1x")
            nc.any.memset(ones_1x[:], 1.0)
            ident = wts.tile([128, 128], bf, tag="ident")
            make_identity(nc, ident)

            # ---- persistent activations ----
            q_sb = [pers.tile([128, S], bf, tag=f"q{h}", name=f"q_sb{h}")
                    for h in range(HPC)]
            k_sb = pers.tile([128, S], bf, tag="k")
            vt_sb = pers.tile([128, S], bf, tag="vt")  # V^T [d, s]
            v_sb = pers.tile([128, S], bf, tag="v")   # [s_in_blk, 16*128 d]

            # gathered activations for phase 3: all 32 heads, my seq group
            g_sb = pers.tile([128, NH, 512], bf, tag="g_sb")

            # ---- collective buffers (parity pair for cross-rep overlap) ----
            # a2a_in rows [d*128,(d+1)*128) = my 4 heads for seq group d>>1
            # a2a_out rows [c*128,(c+1)*128) = core c's 4 heads for my group
            a2a_in = [dram.tile([N_CORES * 128, HPC, 512], bf,
                                tag=f"a2ai{p}", name=f"a2a_in{p}")
                      for p in range(2)]
            a2a_out = [dram.tile([N_CORES * 128, HPC, 512], bf,
                                 tag=f"a2ao{p}", name=f"a2a_out{p}")
                       for p in range(2)]

            # ================= Phase 1: QKV projections =================
            def rope_evict(ps, dst, sc_i):
                """ps: [128,512] f32 PSUM (X^T-space proj), dst bf16 cols."""
                sl = bass.ts(sc_i, 512)
                rot = work.tile([128, 512], f32, tag="rot", bufs=2)
                t1 = work.tile([128, 512], f32, tag="t1", bufs=2)
                # gpsimd has no PSUM port: PSUM-reading ops stay on
                # DVE/ACT, SBUF-only ops can offload to gpsimd
                eng = nc.gpsimd if ROPE_SPLIT else nc.vector
                if ROPE_ACT:
                    nc.scalar.activation(rot[0:64, :], ps[64:128, :],
                                         AF.Copy, scale=-1.0)
                    nc.scalar.copy(rot[64:128, :], ps[0:64, :])
                else:
                    nc.vector.tensor_scalar_mul(rot[0:64, :],
                                                ps[64:128, :], -1.0)
                    nc.vector.tensor_copy(rot[64:128, :], ps[0:64, :])
                nc.vector.tensor_mul(t1[:], ps[:], cos_sb[:, sl])
                eng.tensor_mul(rot[:], rot[:], sin_sb[:, sl])
                eng.tensor_add(dst[:, sl], t1[:], rot[:])

            XG = XG_N   # hc chunks fetched per DMA

            from contextlib import nullcontext

            def proj_chunk(sc_i, pool=None):
                ctx = (nullcontext(pool) if pool is not None else
                       tc.tile_pool(name="ps1", bufs=1, space="PSUM"))
                with ctx as ps1:
                    q_ps = [ps1.tile([128, 512], f32, tag=f"psq{h}",
                                     name=f"q_ps{h}")
                            for h in range(HPC)]
                    k_ps = ps1.tile([128, 512], f32, tag="psk")
                    v_ps = ps1.tile([128, 512], f32, tag="psv")
                    for hg in range(HC // XG):
                        xt_t = xtp.tile([128, XG, 512], bf, tag="xt",
                                        bufs=XTP_BUFS)
                        deng = (nc.scalar if DMA_ALT and hg % 2 else
                                nc.sync)
                        deng.dma_start(
                            xt_t[:],
                            xt[:, sc_i, bass.ts(hg, XG), :])
                        for hx in range(XG):
                            hc = hg * XG + hx
                            st = hc == 0
                            sp = hc == HC - 1
                            for h in range(HPC):
                                nc.tensor.matmul(
                                    q_ps[h][:],
                                    wq_sb[:, hc, bass.ts(h, 128)],
                                    xt_t[:, hx, :], start=st, stop=sp)
                            nc.tensor.matmul(k_ps[:], wk_sb[:, hc, :],
                                             xt_t[:, hx, :],
                                             start=st, stop=sp)
                            nc.tensor.matmul(v_ps[:], wv_sb[:, hc, :],
                                             xt_t[:, hx, :],
                                             start=st, stop=sp)
                    for h in range(HPC):
                        rope_evict(q_ps[h], q_sb[h], sc_i)
                    rope_evict(k_ps, k_sb, sc_i)
                    nc.scalar.copy(vt_sb[:, bass.ts(sc_i, 512)], v_ps[:])
                    for sb_i in range(4):
                        tr_ps = ps1.tile([128, 128], bf, tag="ptr",
                                         bufs=TR_BUFS, name="tr_ps")
                        nc.tensor.transpose(
                            tr_ps[:],
                            vt_sb[:, bass.ds(sc_i * 512 + sb_i * 128, 128)],
                            ident[:])
                        nc.scalar.copy(
                            v_sb[:, bass.ds(sc_i * 512 + sb_i * 128, 128)],
                            tr_ps[:])

            def phase1():
                if PS1_HOIST:
                    with tc.tile_pool(name="ps1", bufs=1,
                                      space="PSUM") as ps1:
                        for sc_i in range(SC):
                            proj_chunk(sc_i, pool=ps1)
                else:
                    for sc_i in range(SC):
                        proj_chunk(sc_i)

            # ================= Phase 2: attention (pipelined) ============
            def a2a_issue(par=0):
                if single:
                    nc.sync.dma_start(a2a_out[par][:], a2a_in[par][:])
                    return
                nc.gpsimd.collective_compute(
                    "AllToAll", mybir.AluOpType.bypass,
                    replica_groups=[list(range(N_CORES))],
                    ins=[a2a_in[par].opt()],
                    outs=[a2a_out[par].opt()],
                )

            def attn_tasks(qts, par=0):
                gw = SPS // 512          # kv blocks per task
                LA = (4 if LA_DEEP else 3) if gw == 1 else 1
                if ROWSUM == "pe" and not (PE_LA3 or RS_BCAST):
                    LA = min(LA, 2)      # rs1 tiles take 2 PSUM banks
                with tc.tile_pool(name="ps2", bufs=1, space="PSUM") as ps2:
                    # flat task list: one task = gw kv blocks of one (qt,h)
                    tasks = []
                    for qt in qts:
                        nkb = 4 * (qt + 1)
                        for h in range(HPC):
                            for kb0 in range(0, nkb, gw):
                                tasks.append((qt, h, kb0, nkb))

                    state = {}   # (qt,h) -> dict(out_ps, acc, rs1)
                    oq_sb = {}   # qt -> staging tile

                    def issue_scores(t):
                        qt, h, kb0, _ = t
                        s_ps = ps2.tile([128, SPS], f32, tag="s",
                                        bufs=LA + 1, name="s_ps")
                        for half in range(gw):
                            nc.tensor.matmul(
                                s_ps[:, bass.ts(half, 512)],
                                k_sb[:, bass.ts(kb0 + half, 128)],
                                q_sb[h][:, bass.ts(qt, 512)],
                                start=True, stop=True)
                        return s_ps

                    def ship(qt):
                        # stage my heads' outputs for seq group qt to the
                        # two consumer cores (2qt, 2qt+1)
                        for d in (2 * qt, 2 * qt + 1):
                            nc.sync.dma_start(
                                a2a_in[par][bass.ds(d * 128, 128), :, :],
                                oq_sb[qt][:])

                    def process(t, s_ps):
                        qt, h, kb0, nkb = t
                        key = (qt, h)
                        if kb0 == 0:
                            st = {"out": ps2.tile([128, 512], f32, tag="out",
                                                  bufs=2, name="out_ps")}
                            if ROWSUM == "pe":
                                st["rs1"] = ps2.tile([1, 512], f32,
                                                     tag="rs1", bufs=2,
                                                     name="rs1_ps")
                            else:
                                st["acc"] = work.tile([128, 512], f32,
                                                      tag="pacc", bufs=2,
                                                      name="acc")
                            state[key] = st
                        st = state[key]
                        p_sb = ppool.tile([128, SPS], bf, tag="p",
                                          bufs=LA + (1 if P_SLIM else 2))
                        nc.scalar.activation(p_sb[:], s_ps[:], AF.Exp,
                                             scale=scale)
                        for half in range(gw):
                            kb = kb0 + half
                            ph = p_sb[:, bass.ts(half, 512)]
                            if kb >= 4 * qt:
                                j = kb - 4 * qt
                                eng = (nc.vector if MASK_ENGINE == "vector"
                                       else nc.gpsimd)
                                eng.tensor_mul(
                                    ph, ph,
                                    msk_sb[:, 512 - 128 * j:1024 - 128 * j])
                            nc.tensor.matmul(
                                st["out"][:], v_sb[:, bass.ts(kb, 128)],
                                ph, start=kb == 0, stop=kb == nkb - 1)
                            if ROWSUM == "pe":
                                nc.tensor.matmul(
                                    st["rs1"][:], ones_col[:], ph,
                                    start=kb == 0, stop=kb == nkb - 1)
                            else:
                                if ROWSUM == "gps":
                                    eng = nc.gpsimd
                                elif ROWSUM == "dve":
                                    eng = nc.vector
                                else:   # mix: DVE is ~2x gpsimd on f32
                                    eng = (nc.gpsimd if kb % 3 == 2
                                           else nc.vector)
                                if kb == 0:
                                    eng.tensor_copy(st["acc"][:], ph)
                                else:
                                    eng.tensor_add(st["acc"][:],
                                                   st["acc"][:], ph)
                        if kb0 == nkb - gw:         # group done -> finalize
                            del state[key]
                            return lambda: finalize(qt, h, st)
                        return None

                    def finalize(qt, h, st):
                        if qt not in oq_sb:
                            oq_sb[qt] = work.tile([128, HPC, 512], bf,
                                                  tag="oqs", bufs=2,
                                                  name="oq_sb")
                        if ROWSUM == "pe" and RS_BCAST:
                            rb1_sb = work.tile([1, 512], f32,
                                               tag="rs1_sb", bufs=2,
                                               name="rb1_sb")
                            nc.vector.reciprocal(rb1_sb[:], st["rs1"][:])
                            rb_sb = work.tile([128, 512], f32,
                                              tag="rb_sb", bufs=2)
                            nc.gpsimd.partition_broadcast(rb_sb[:],
                                                          rb1_sb[:])
                            nc.vector.tensor_mul(oq_sb[qt][:, h, :],
                                                 st["out"][:], rb_sb[:])
                            if h == HPC - 1:
                                ship(qt)
                            return
                        if ROWSUM == "pe" and PE_LA3:
                            # borrow a scores slot (same shape) so no
                            # dedicated rs bank is needed and LA stays 3
                            rs_ps = ps2.tile([128, SPS], f32, tag="s",
                                             bufs=LA + 1, name="rs_ps")
                        else:
                            rs_ps = ps2.tile([128, 512], f32, tag="rs",
                                             bufs=1 if (LA_DEEP or
                                                        ROWSUM == "pe")
                                             else 2,
                                             name="rs_ps")
                        if ROWSUM == "pe":
                            rs1_sb = work.tile([1, 512], f32,
                                               tag="rs1_sb", bufs=2)
                            nc.scalar.copy(rs1_sb[:], st["rs1"][:])
                            nc.tensor.matmul(rs_ps[:], ones_1x[:],
                                             rs1_sb[:],
                                             start=True, stop=True)
                        else:
                            nc.tensor.matmul(rs_ps[:], ones128[:],
                                             st["acc"][:],
                                             start=True, stop=True)
                        rb_sb = work.tile([128, 512], f32, tag="rb_sb",
                                          bufs=2)
                        nc.vector.reciprocal(rb_sb[:], rs_ps[:])
                        nc.vector.tensor_mul(oq_sb[qt][:, h, :],
                                             st["out"][:], rb_sb[:])
                        if h == HPC - 1:
                            ship(qt)

                    from collections import deque
                    pending = deque()
                    fin = [None]

                    def run_proc(args):
                        f = process(*args)
                        if not FIN_DEFER:
                            if f is not None:
                                f()
                            return
                        if fin[0] is not None:
                            fin[0]()
                        fin[0] = f

                    for t in tasks:
                        pending.append((t, issue_scores(t)))
                        if len(pending) > LA:
                            run_proc(pending.popleft())
                    while pending:
                        run_proc(pending.popleft())
                    if fin[0] is not None:
                        fin[0]()

            def phase2(do_cc=True, par=0):
                attn_tasks(list(range(SC)), par=par)
                if do_cc:
                    a2a_issue(par)

            def phase12(do_cc=True, par=0):
                for sc_i in range(SC):
                    proj_chunk(sc_i)
                    attn_tasks([sc_i], par=par)
                if do_cc:
                    a2a_issue(par)

            # ====== Phase 3: (seq-group, feature-half)-sharded out-proj ==
            def phase3_copy(par=0):
                # land gathered activations in SBUF: all 32 heads for my
                # 512-seq group
                # g_sb copies wait on the collective — issue them on the
                # scalar queue so wop weight prefetches (sync queue) are
                # not head-of-line blocked behind them during the A2A
                for c in range(N_CORES):
                    nc.scalar.dma_start(
                        g_sb[:, bass.ds(c * HPC, HPC), :],
                        a2a_out[par][bass.ds(c * 128, 128), :, :])

            def phase3(par=0, skip_copy=False):
                if not skip_copy:
                    phase3_copy(par)
                with tc.tile_pool(name="ps3", bufs=2, space="PSUM") as ps3:
                    for ft in range(FT):
                        wop = gpool.tile([128, NH, 128], bf, tag="wop",
                                         bufs=WOP_BUFS)
                        nc.sync.dma_start(wop[:], wo[:, ft, :, :])
                        out_ps = ps3.tile([128, 512], f32, tag="op",
                                          name="out_ps")
                        for h in range(NH):
                            nc.tensor.matmul(
                                out_ps[:], wop[:, h, :], g_sb[:, h, :],
                                start=h == 0, stop=h == NH - 1)
                        if Y_DIRECT:
                            nc.scalar.dma_start(y[bass.ts(ft, 128), :],
                                                out_ps[:])
                            continue
                        y_sb = work.tile([128, 512], f32, tag="y_sb",
                                         bufs=2)
                        if ft % 2 == 0:
                            nc.vector.tensor_copy(y_sb[:], out_ps[:])
                        else:
                            nc.scalar.copy(y_sb[:], out_ps[:])
                        # y on the gpsimd queue: keeps the sync queue
                        # free so the next rep's xt prefetch isn't blocked
                        # behind wop slot-waits
                        nc.gpsimd.dma_start(y[bass.ts(ft, 128), :], y_sb[:])

            phase_fns = {1: phase1, 2: phase2, 3: phase3}

            if not timing and reps_phases is not None:
                # static-unrolled per-phase timing builds
                ph = set(reps_phases)
                if 1 not in ph:
                    for t in [k_sb, v_sb] + q_sb:
                        nc.any.memset(t[:], 0.0)
                if 2 not in ph and 3 in ph:
                    z_sb = work.tile([128, HPC, 512], bf, tag="oqs",
                                     bufs=2, name="z_sb")
                    nc.any.memset(z_sb[:], 0.0)
                    for c in range(N_CORES):
                        nc.sync.dma_start(
                            a2a_out[0][bass.ds(c * 128, 128), :, :],
                            z_sb[:])
                for _rep in range(reps):
                    if 1 in ph and 2 in ph and FUSE12:
                        phase12(do_cc=not skip_cc)
                    else:
                        if 1 in ph:
                            phase1()
                        if 2 in ph:
                            phase2(do_cc=not skip_cc)
                    if 3 in ph:
                        phase3()
            elif not timing:
                # PIPE: emit phase3 of rep r after phase1/2 of rep r+1 so
                # the AllToAll flies behind the next rep's projection and
                # attention compute (parity-double-buffered a2a tiles).
                # reps=1 emission is identical either way.
                if PIPE:
                    for _rep in range(reps):
                        par = _rep % 2
                        if FUSE12:
                            phase12(do_cc=not skip_cc, par=par)
                        else:
                            phase1()
                            phase2(do_cc=not skip_cc, par=par)
                        if _rep > 0:
                            phase3(par=1 - par)
                    phase3(par=(reps - 1) % 2)
                else:
                    for _rep in range(reps):
                        if FUSE12:
                            phase12(do_cc=not skip_cc)
                        else:
                            phase1()
                            phase2(do_cc=not skip_cc)
                        phase3()
            else:
                # Collectives cannot sit inside a For_i loop: phase2 runs
                # with do_cc=False in the loop; the A2A is issued once
                # after it. Skipped producer phases get cheap inits so
                # consumers' tiles exist (values irrelevant for timing).
                if phases_only:
                    if 1 not in phases:
                        for t in [k_sb, v_sb] + q_sb:
                            nc.any.memset(t[:], 0.0)
                    if 2 not in phases and 3 in phases:
                        z_sb = work.tile([128, HPC, 512], bf, tag="oqs",
                                         bufs=2, name="z_sb")
                        nc.any.memset(z_sb[:], 0.0)
                        for c in range(N_CORES):
                            nc.sync.dma_start(
                                a2a_out[0][bass.ds(c * 128, 128), :, :],
                                z_sb[:])
                loop_body = []
                for p in phases:
                    if p == 2:
                        loop_body.append(lambda: phase2(do_cc=False))
                    else:
                        loop_body.append(phase_fns[p])
                for p in (1, 2, 3):
                    if (p not in phases and p < min(phases)
                            and not phases_only):
                        phase_fns[p]()
                if timing == 1:
                    for fn in loop_body:
                        fn()
                else:
                    with tc.For_i(0, int(timing), 1):
                        for fn in loop_body:
                            fn()
                if 2 in phases:
                    a2a_issue()
                for p in (1, 2, 3):
                    if p not in phases and p > max(phases) and not phases_only:
                        phase_fns[p]()

    nc.compile()
    return nc


class BassExec:
    """Build-once, run-many SPMD executor over the axon PJRT path.

    Modeled on concourse.bass2jax.run_bass_via_pjrt, but keeps the jitted
    callable so repeated executions skip re-tracing/re-compiling.
    """

    def __init__(self, nc, n_cores):
        import jax
        from jax.sharding import Mesh, PartitionSpec, NamedSharding
        from jax.experimental.shard_map import shard_map
        from concourse import bass2jax
        from concourse.bass2jax import _bass_exec_p, partition_id_tensor

        bass2jax.install_neuronx_cc_hook()
        self.jax = jax
        self.nc = nc
        self.n_cores = n_cores
        partition_name = (nc.partition_id_tensor.name
                          if nc.partition_id_tensor else None)
        in_names, out_names, out_avals, zero_outs = [], [], [], []
        for alloc in nc.m.functions[0].allocations:
            if not isinstance(alloc, mybir.MemoryLocationSet):
                continue
            name = alloc.memorylocations[0].name
            if alloc.kind == "ExternalInput":
                if name != partition_name:
                    in_names.append(name)
            elif alloc.kind == "ExternalOutput":
                out_names.append(name)
                shape = tuple(alloc.tensor_shape)
                dtype = mybir.dt.np(alloc.dtype)
                out_avals.append(jax.core.ShapedArray(shape, dtype))
                zero_outs.append(np.zeros(shape, dtype))
        self.in_names, self.out_names = in_names, out_names
        self.out_avals, self.zero_outs = out_avals, zero_outs
        n_params = len(in_names)
        n_outs = len(out_avals)
        all_in_names = list(in_names) + list(out_names)
        if partition_name is not None:
            all_in_names.append(partition_name)

        def _body(*args):
            operands = list(args)
            if partition_name is not None:
                operands.append(partition_id_tensor())
            outs = _bass_exec_p.bind(
                *operands,
                out_avals=tuple(out_avals),
                in_names=tuple(all_in_names),
                out_names=tuple(out_names),
                lowering_input_output_aliases=(),
                sim_require_finite=True,
                sim_require_nnan=True,
                nc=nc,
            )
            return tuple(outs)

        devices = jax.devices()[:n_cores]
        self.mesh = Mesh(np.asarray(devices), ("core",))
        in_specs = (PartitionSpec("core"),) * (n_params + n_outs)
        out_specs = (PartitionSpec("core"),) * n_outs
        donate = tuple(range(n_params, n_params + n_outs))
        self.sharded = jax.jit(
            shard_map(_body, mesh=self.mesh, in_specs=in_specs,
                      out_specs=out_specs, check_rep=False),
            donate_argnums=donate, keep_unused=True,
        )
        self.sharding = NamedSharding(self.mesh, PartitionSpec("core"))

    def put_inputs(self, in_maps):
        concat = [np.concatenate([np.asarray(in_maps[c][n])
                                  for c in range(self.n_cores)], axis=0)
                  for n in self.in_names]
        return [self.jax.device_put(a, self.sharding) for a in concat]

    def zeros_dev(self):
        return [self.jax.device_put(
            np.zeros((self.n_cores * z.shape[0], *z.shape[1:]), z.dtype),
            self.sharding) for z in self.zero_outs]

    def run(self, ins_dev):
        outs = self.sharded(*ins_dev, *self.zeros_dev())
        self.jax.block_until_ready(outs)
        return outs

    def results(self, outs):
        return [{name: np.asarray(outs[i]).reshape(
                    self.n_cores, *self.out_avals[i].shape)[c]
                 for i, name in enumerate(self.out_names)}
                for c in range(self.n_cores)]


_CACHE = {}


def _get_exec():
    if "exec" not in _CACHE:
        _CACHE["exec"] = BassExec(build_nc(), N_CORES)
    return _CACHE["exec"]


def make_in_maps(hidden_states, position_ids, Wq, Wk, Wv, Wo):
    X = np.asarray(hidden_states)[0]          # [S, H] f32
    pos = np.asarray(position_ids)[0]                      # [S]
    inv = 1.0 / (ROPE_THETA ** (np.arange(0, D, 2, dtype=np.float32) / D))
    inv_full = np.concatenate([inv, inv]).astype(np.float32)   # [128]
    # fp32 product (matches reference's fp32 freqs), then exact range
    # reduction to [-pi, pi) where the ACT Sin unit is accurate
    prod = (pos[None, :].astype(np.float32)
            * inv_full[:, None]).astype(np.float64)
    tp = 2 * np.pi
    fsin = (np.mod(prod + np.pi, tp) - np.pi).astype(np.float32)
    fcos = (np.mod(prod + np.pi / 2 + np.pi, tp) - np.pi).astype(np.float32)

    t = np.arange(1024)[None, :]
    k = np.arange(128)[:, None]
    msk = (t >= k + 512).astype(ml_dtypes.bfloat16)        # [128, 1024]

    xt = np.ascontiguousarray(
        X.reshape(SC, 512, HC, 128).transpose(3, 0, 2, 1)
    ).astype(ml_dtypes.bfloat16)                       # [128, SC, HC, 512]

    in_maps = []
    for c in range(N_CORES):
        wq_c = np.asarray(Wq)[:, c * QO:(c + 1) * QO]       # [H, 512]
        wk_c = np.asarray(Wk)[:, c * D:(c + 1) * D]         # [H, 128]
        wv_c = np.asarray(Wv)[:, c * D:(c + 1) * D]
        fh = c & 1
        wo_c = np.asarray(Wo)[:, fh * FH:(fh + 1) * FH]     # [H, 2048]
        in_maps.append({
            "xt": xt,
            "wq": np.ascontiguousarray(
                wq_c.reshape(HC, 128, QO).transpose(1, 0, 2)
            ).astype(ml_dtypes.bfloat16),
            "wk": np.ascontiguousarray(
                wk_c.reshape(HC, 128, D).transpose(1, 0, 2)
            ).astype(ml_dtypes.bfloat16),
            "wv": np.ascontiguousarray(
                wv_c.reshape(HC, 128, D).transpose(1, 0, 2)
            ).astype(ml_dtypes.bfloat16),
            "wo": np.ascontiguousarray(
                wo_c.reshape(NH, 128, FT, 128).transpose(1, 2, 0, 3)
            ).astype(ml_dtypes.bfloat16),              # [128, FT, NH, 128]
            "fsin": fsin,
            "fcos": fcos,
            "msk": np.ascontiguousarray(msk),
        })
    return in_maps


def assemble_output(results):
    # results[c]["y"]: [2048, 512] = Y^T rows fh*2048.., cols sg*512..
    # with sg = c>>1, fh = c&1
    final_t = np.empty((H, S), np.float32)
    for c in range(N_CORES):
        sg, fh = c >> 1, c & 1
        final_t[fh * FH:(fh + 1) * FH, sg * 512:(sg + 1) * 512] = \
            results[c]["y"]
    return np.ascontiguousarray(final_t.T)[None].astype(np.float32)


def kernel(hidden_states, position_ids, Wq, Wk, Wv, Wo):
    ex = _get_exec()
    in_maps = make_in_maps(hidden_states, position_ids, Wq, Wk, Wv, Wo)
    outs = ex.run(ex.put_inputs(in_maps))
    return assemble_output(ex.results(outs))


if __name__ == "__main__":
    rng = np.random.default_rng(0)
    hs = rng.standard_normal((1, S, H)).astype(np.float32)
    pid = np.broadcast_to(np.arange(S, dtype=np.int32), (1, S))
    Wq_ = (rng.standard_normal((H, NH * D)) * 0.02).astype(np.float32)
    Wk_ = (rng.standard_normal((H, NKV * D)) * 0.02).astype(np.float32)
    Wv_ = (rng.standard_normal((H, NKV * D)) * 0.02).astype(np.float32)
    Wo_ = (rng.standard_normal((NH * D, H)) * 0.02).astype(np.float32)
    out = kernel(hs, pid, Wq_, Wk_, Wv_, Wo_)
    print("out", out.shape, out.dtype, out[0, :2, :4])


# revision 41
# speedup vs baseline: 1.0126x; 1.0126x over previous
"""Trainium2 Bass kernel for a GQA attention block (NeuronAttentionBase).

Shapes: B=1, S=2048, H=4096, NH=32 query heads, NKV=8 kv heads, D=128.
Sharding: tensor-parallel across heads on 8 NeuronCores — 4 query heads +
1 kv head per core; Wq/Wk/Wv column-sharded.

Out-projection is sharded 2D over (seq-group, feature-half): core c
computes Y^T[fh*2048:(fh+1)*2048, sg*512:(sg+1)*512] with sg=c>>1,
fh=c&1. Feeding it needs each core's per-head attention output only for
ONE 512-seq group on each consumer, so a single 4MB AllToAll replaces
the previous 2x8MB AllGather (14MB received per core -> 3.7MB): slot d
of the A2A input carries this core's 4 heads for seq group d>>1. Wo is
streamed from HBM during phase 3 (16.8MB over ~110us of PE work)
instead of being SBUF-resident.

All compute runs in "transposed space" (activations stored as [feature,
seq] tiles) so no on-device transposes are needed anywhere:
  Q^T/K^T  = matmul(lhsT=W, rhs=X^T)        -> [d, s]
  V        = matmul(lhsT=X^T_blk, rhs=Wv)    -> [s, d]   (natural)
  S^T      = matmul(lhsT=K^T_blk, rhs=Q^T)   -> [k, q]
  P~^T     = exp(S^T/sqrt(D)) * causal_mask  (no max subtraction; scores
             are O(10) for this distribution so fp32 exp is safe)
  OUT^T    = matmul(lhsT=V_blk, rhs=P~^T)    -> [d, q]  (+ rowsums;
             normalization applied on PSUM eviction)
  Y^T_blk  = matmul(lhsT=Wo_ft, rhs=alltoall(OUT^T)) -> [128, 512]

Phase 2 is software-pipelined: the scores matmuls for block-group g+1
are issued before the exp/PV of group g, so the in-order PE queue never
stalls on the ACT engine's exp latency.
"""

import math
import os

import numpy as np
import ml_dtypes


def _env(name, default):
    v = os.environ.get(f"K_{name}")
    if v is None:
        return default
    if isinstance(default, bool):
        return v not in ("0", "false", "False")
    if isinstance(default, int):
        return int(v)
    return v

import concourse.bass as bass
import concourse.mybir as mybir
import concourse.tile as tile
from concourse import bacc
from concourse.masks import make_identity

N_CORES = 8
S = 2048
H = 4096
NH, NKV, D = 32, 8, 128
HPC = NH // N_CORES          # query heads per core = 4
QO = HPC * D                 # per-core Wq out cols = 512
FH = H // 2                  # feature half = 2048 (phase3 cols per core)
FT = FH // 128               # 16 feature tiles per core
HC = H // 128                # 32 contraction chunks
SC = S // 512                # 4 seq chunks of 512
SB = S // 128                # 16 seq blocks of 128
ROPE_THETA = 10000.0
MASK_ENGINE = _env("MASK_ENGINE", "gpsimd")   # "vector" | "gpsimd"
ROWSUM = _env("ROWSUM", "pe")  # "pe" ones-matmul in PSUM (no cross-engine
                             # stall) | "mix" 2:1 dve/gps | "gps" | "dve"
SPS = _env("SPS", 512)       # scores tile width: 512 (lookahead 2) | 1024
ROPE_SPLIT = _env("ROPE_SPLIT", False)  # rope eviction across DVE + gpsimd
FIN_DEFER = _env("FIN_DEFER", False)
Y_DIRECT = _env("Y_DIRECT", False)
XG_N = _env("XG_N", 8)       # hc chunks per phase1 DMA
XTP_BUFS = _env("XTP_BUFS", 3)   # phase1 xt prefetch depth
LA_DEEP = _env("LA_DEEP", False)
P_SLIM = _env("P_SLIM", False)
DMA_ALT = _env("DMA_ALT", False)
TR_BUFS = _env("TR_BUFS", 2)
PE_LA3 = _env("PE_LA3", False)
ROPE_ACT = _env("ROPE_ACT", True)
RS_BCAST = _env("RS_BCAST", False)
WOP_BUFS = _env("WOP_BUFS", 3)   # phase3 Wo stream prefetch depth
FUSE12 = _env("FUSE12", False)   # interleave proj/attn per seq chunk
PS1_HOIST = _env("PS1_HOIST", True)  # one ps1 PSUM pool for all of phase1
PIPE = _env("PIPE", True)   # overlap rep r's A2A+phase3 with rep r+1
DTRIM = _env("DTRIM", True)   # trim masked columns of diagonal blocks:
                             # scores matmul + exp restricted to valid
                             # cols, masked prefix zeroed on gpsimd,
                             # mask mul shrunk to the triangle block

bf = mybir.dt.bfloat16
f32 = mybir.dt.float32
AF = mybir.ActivationFunctionType


def build_nc(timing=False, phases=(1, 2, 3), single=False,
             phases_only=False, reps=1, skip_cc=False, reps_phases=None):
    """timing=R (int>0) wraps the compute phases (not the collectives)
    in a hardware For_i loop, so device time per iteration can be
    measured as (wall(R) - wall(1)) / (R-1).

    reps=R (static unroll, timing=False path only) repeats the FULL
    kernel — compute phases AND the AllToAll collective — R times in
    one NEFF. (wall(reps=R) - wall(reps=1)) / (R-1) is the complete
    per-kernel device time including collectives."""
    nc = bacc.Bacc(None, target_bir_lowering=False, debug=False,
                   num_devices=1 if single else N_CORES)
    # xt laid out seq-chunk-major so each phase-1 fetch is contiguous
    # per partition; wo laid out feature-tile-major likewise
    xt = nc.dram_tensor("xt", [128, SC, HC, 512], bf, kind="ExternalInput")
    wq = nc.dram_tensor("wq", [128, HC, QO], bf, kind="ExternalInput")
    wk = nc.dram_tensor("wk", [128, HC, D], bf, kind="ExternalInput")
    wv = nc.dram_tensor("wv", [128, HC, D], bf, kind="ExternalInput")
    wo = nc.dram_tensor("wo", [128, FT, NH, 128], bf, kind="ExternalInput")
    fsin = nc.dram_tensor("fsin", [128, S], f32, kind="ExternalInput")
    fcos = nc.dram_tensor("fcos", [128, S], f32, kind="ExternalInput")
    msk = nc.dram_tensor("msk", [128, 1024], bf, kind="ExternalInput")
    y = nc.dram_tensor("y", [FH, 512], f32, kind="ExternalOutput")

    scale = 1.0 / math.sqrt(D)

    with tile.TileContext(nc) as tc:
        with (
            tc.tile_pool(name="wts", bufs=1) as wts,
            tc.tile_pool(name="pers", bufs=1) as pers,
            tc.tile_pool(name="xtp", bufs=3) as xtp,
            tc.tile_pool(name="work", bufs=3) as work,
            tc.tile_pool(name="ppool", bufs=3) as ppool,
            tc.tile_pool(name="gpool", bufs=2) as gpool,
            tc.tile_pool(name="dram", bufs=1, space="DRAM") as dram,
        ):
            # ---- resident weights ----
            wq_sb = wts.tile([128, HC, QO], bf, tag="wq")
            wk_sb = wts.tile([128, HC, D], bf, tag="wk")
            wv_sb = wts.tile([128, HC, D], bf, tag="wv")
            nc.sync.dma_start(wq_sb[:], wq[:])
            nc.sync.dma_start(wk_sb[:], wk[:])
            nc.sync.dma_start(wv_sb[:], wv[:])

            msk_sb = wts.tile([128, 1024], bf, tag="msk")
            nc.sync.dma_start(msk_sb[:], msk[:])

            # ---- RoPE cos/sin tables (args pre-reduced to [-pi, pi)),
            # computed in 512-col chunks through the shared work tiles ----
            cos_sb = pers.tile([128, S], f32, tag="cos")
            sin_sb = pers.tile([128, S], f32, tag="sin")
            for sc_i in range(SC):
                sl = bass.ts(sc_i, 512)
                ftmp = work.tile([128, 512], f32, tag="rot", bufs=2,
                                 name="ftmp")
                nc.sync.dma_start(ftmp[:], fsin[:, sl])
                nc.scalar.activation(sin_sb[:, sl], ftmp[:], AF.Sin)
                ftmp2 = work.tile([128, 512], f32, tag="t1", bufs=2,
                                  name="ftmp2")
                nc.sync.dma_start(ftmp2[:], fcos[:, sl])
                nc.scalar.activation(cos_sb[:, sl], ftmp2[:], AF.Sin)

            # ---- constants ----
            ones128 = wts.tile([128, 128], f32, tag="ones128")
            nc.any.memset(ones128[:], 1.0)
            ones_col = wts.tile([128, 1], bf, tag="ones_col")
            nc.any.memset(ones_col[:], 1.0)
            ones_1x = wts.tile([1, 128], f32, tag="ones_1x")
            nc.any.memset(ones_1x[:], 1.0)
            ident = wts.tile([128, 128], bf, tag="ident")
            make_identity(nc, ident)

            # ---- persistent activations ----
            q_sb = [pers.tile([128, S], bf, tag=f"q{h}", name=f"q_sb{h}")
                    for h in range(HPC)]
            k_sb = pers.tile([128, S], bf, tag="k")
            vt_sb = pers.tile([128, S], bf, tag="vt")  # V^T [d, s]
            v_sb = pers.tile([128, S], bf, tag="v")   # [s_in_blk, 16*128 d]

            # gathered activations for phase 3: all 32 heads, my seq group
            g_sb = pers.tile([128, NH, 512], bf, tag="g_sb")

            # ---- collective buffers (parity pair for cross-rep overlap) ----
            # a2a_in rows [d*128,(d+1)*128) = my 4 heads for seq group d>>1
            # a2a_out rows [c*128,(c+1)*128) = core c's 4 heads for my group
            a2a_in = [dram.tile([N_CORES * 128, HPC, 512], bf,
                                tag=f"a2ai{p}", name=f"a2a_in{p}")
                      for p in range(2)]
            a2a_out = [dram.tile([N_CORES * 128, HPC, 512], bf,
                                 tag=f"a2ao{p}", name=f"a2a_out{p}")
                       for p in range(2)]

            # ================= Phase 1: QKV projections =================
            def rope_evict(ps, dst, sc_i):
                """ps: [128,512] f32 PSUM (X^T-space proj), dst bf16 cols."""
                sl = bass.ts(sc_i, 512)
                rot = work.tile([128, 512], f32, tag="rot", bufs=2)
                t1 = work.tile([128, 512], f32, tag="t1", bufs=2)
                # gpsimd has no PSUM port: PSUM-reading ops stay on
                # DVE/ACT, SBUF-only ops can offload to gpsimd
                eng = nc.gpsimd if ROPE_SPLIT else nc.vector
                if ROPE_ACT:
                    nc.scalar.activation(rot[0:64, :], ps[64:128, :],
                                         AF.Copy, scale=-1.0)
                    nc.scalar.copy(rot[64:128, :], ps[0:64, :])
                else:
                    nc.vector.tensor_scalar_mul(rot[0:64, :],
                                                ps[64:128, :], -1.0)
                    nc.vector.tensor_copy(rot[64:128, :], ps[0:64, :])
                nc.vector.tensor_mul(t1[:], ps[:], cos_sb[:, sl])
                eng.tensor_mul(rot[:], rot[:], sin_sb[:, sl])
                eng.tensor_add(dst[:, sl], t1[:], rot[:])

            XG = XG_N   # hc chunks fetched per DMA

            from contextlib import nullcontext

            def proj_chunk(sc_i, pool=None):
                ctx = (nullcontext(pool) if pool is not None else
                       tc.tile_pool(name="ps1", bufs=1, space="PSUM"))
                with ctx as ps1:
                    q_ps = [ps1.tile([128, 512], f32, tag=f"psq{h}",
                                     name=f"q_ps{h}")
                            for h in range(HPC)]
                    k_ps = ps1.tile([128, 512], f32, tag="psk")
                    v_ps = ps1.tile([128, 512], f32, tag="psv")
                    for hg in range(HC // XG):
                        xt_t = xtp.tile([128, XG, 512], bf, tag="xt",
                                        bufs=XTP_BUFS)
                        deng = (nc.scalar if DMA_ALT and hg % 2 else
                                nc.sync)
                        deng.dma_start(
                            xt_t[:],
                            xt[:, sc_i, bass.ts(hg, XG), :])
                        for hx in range(XG):
                            hc = hg * XG + hx
                            st = hc == 0
                            sp = hc == HC - 1
                            for h in range(HPC):
                                nc.tensor.matmul(
                                    q_ps[h][:],
                                    wq_sb[:, hc, bass.ts(h, 128)],
                                    xt_t[:, hx, :], start=st, stop=sp)
                            nc.tensor.matmul(k_ps[:], wk_sb[:, hc, :],
                                             xt_t[:, hx, :],
                                             start=st, stop=sp)
                            nc.tensor.matmul(v_ps[:], wv_sb[:, hc, :],
                                             xt_t[:, hx, :],
                                             start=st, stop=sp)
                    for h in range(HPC):
                        rope_evict(q_ps[h], q_sb[h], sc_i)
                    rope_evict(k_ps, k_sb, sc_i)
                    nc.scalar.copy(vt_sb[:, bass.ts(sc_i, 512)], v_ps[:])
                    for sb_i in range(4):
                        tr_ps = ps1.tile([128, 128], bf, tag="ptr",
                                         bufs=TR_BUFS, name="tr_ps")
                        nc.tensor.transpose(
                            tr_ps[:],
                            vt_sb[:, bass.ds(sc_i * 512 + sb_i * 128, 128)],
                            ident[:])
                        nc.scalar.copy(
                            v_sb[:, bass.ds(sc_i * 512 + sb_i * 128, 128)],
                            tr_ps[:])

            def phase1():
                if PS1_HOIST:
                    with tc.tile_pool(name="ps1", bufs=1,
                                      space="PSUM") as ps1:
                        for sc_i in range(SC):
                            proj_chunk(sc_i, pool=ps1)
                else:
                    for sc_i in range(SC):
                        proj_chunk(sc_i)

            # ================= Phase 2: attention (pipelined) ============
            def a2a_issue(par=0):
                if single:
                    nc.sync.dma_start(a2a_out[par][:], a2a_in[par][:])
                    return
                nc.gpsimd.collective_compute(
                    "AllToAll", mybir.AluOpType.bypass,
                    replica_groups=[list(range(N_CORES))],
                    ins=[a2a_in[par].opt()],
                    outs=[a2a_out[par].opt()],
                )

            def attn_tasks(qts, par=0):
                gw = SPS // 512          # kv blocks per task
                LA = (4 if LA_DEEP else 3) if gw == 1 else 1
                if ROWSUM == "pe" and not (PE_LA3 or RS_BCAST):
                    LA = min(LA, 2)      # rs1 tiles take 2 PSUM banks
                with tc.tile_pool(name="ps2", bufs=1, space="PSUM") as ps2:
                    # flat task list: one task = gw kv blocks of one (qt,h)
                    tasks = []
                    for qt in qts:
                        nkb = 4 * (qt + 1)
                        for h in range(HPC):
                            for kb0 in range(0, nkb, gw):
                                tasks.append((qt, h, kb0, nkb))

                    state = {}   # (qt,h) -> dict(out_ps, acc, rs1)
                    oq_sb = {}   # qt -> staging tile

                    def _trim(qt, kb):
                        # masked column prefix width for diagonal blocks
                        if not DTRIM or gw != 1 or kb < 4 * qt:
                            return 0
                        return 128 * (kb - 4 * qt)

                    def issue_scores(t):
                        qt, h, kb0, _ = t
                        s_ps = ps2.tile([128, SPS], f32, tag="s",
                                        bufs=LA + 1, name="s_ps")
                        for half in range(gw):
                            tr = _trim(qt, kb0 + half)
                            nc.tensor.matmul(
                                s_ps[:, bass.ds(half * 512 + tr, 512 - tr)],
                                k_sb[:, bass.ts(kb0 + half, 128)],
                                q_sb[h][:, bass.ds(qt * 512 + tr, 512 - tr)],
                                start=True, stop=True)
                        return s_ps

                    def ship(qt):
                        # stage my heads' outputs for seq group qt to the
                        # two consumer cores (2qt, 2qt+1)
                        for d in (2 * qt, 2 * qt + 1):
                            nc.sync.dma_start(
                                a2a_in[par][bass.ds(d * 128, 128), :, :],
                                oq_sb[qt][:])

                    def process(t, s_ps):
                        qt, h, kb0, nkb = t
                        key = (qt, h)
                        if kb0 == 0:
                            st = {"out": ps2.tile([128, 512], f32, tag="out",
                                                  bufs=2, name="out_ps")}
                            if ROWSUM == "pe":
                                st["rs1"] = ps2.tile([1, 512], f32,
                                                     tag="rs1", bufs=2,
                                                     name="rs1_ps")
                            else:
                                st["acc"] = work.tile([128, 512], f32,
                                                      tag="pacc", bufs=2,
                                                      name="acc")
                            state[key] = st
                        st = state[key]
                        p_sb = ppool.tile([128, SPS], bf, tag="p",
                                          bufs=LA + (1 if P_SLIM else 2))
                        tr0 = _trim(qt, kb0) if gw == 1 else 0
                        if tr0:
                            # zero the masked prefix (idle gpsimd) so the
                            # full-width PV/rowsum matmuls add zeros there
                            nc.gpsimd.memset(p_sb[:, 0:tr0], 0.0)
                            nc.scalar.activation(
                                p_sb[:, bass.ds(tr0, 512 - tr0)],
                                s_ps[:, bass.ds(tr0, 512 - tr0)],
                                AF.Exp, scale=scale)
                        else:
                            nc.scalar.activation(p_sb[:], s_ps[:], AF.Exp,
                                                 scale=scale)
                        for half in range(gw):
                            kb = kb0 + half
                            ph = p_sb[:, bass.ts(half, 512)]
                            if kb >= 4 * qt:
                                j = kb - 4 * qt
                                eng = (nc.vector if MASK_ENGINE == "vector"
                                       else nc.gpsimd)
                                if DTRIM and gw == 1:
                                    # only the [128,128] triangle block
                                    # needs masking; beyond it msk is 1,
                                    # before it the prefix is zeroed
                                    eng.tensor_mul(
                                        p_sb[:, bass.ds(128 * j, 128)],
                                        p_sb[:, bass.ds(128 * j, 128)],
                                        msk_sb[:, 512:640])
                                else:
                                    eng.tensor_mul(
                                        ph, ph,
                                        msk_sb[:,
                                               512 - 128 * j:1024 - 128 * j])
                            nc.tensor.matmul(
                                st["out"][:], v_sb[:, bass.ts(kb, 128)],
                                ph, start=kb == 0, stop=kb == nkb - 1)
                            if ROWSUM == "pe":
                                nc.tensor.matmul(
                                    st["rs1"][:], ones_col[:], ph,
                                    start=kb == 0, stop=kb == nkb - 1)
                            else:
                                if ROWSUM == "gps":
                                    eng = nc.gpsimd
                                elif ROWSUM == "dve":
                                    eng = nc.vector
                                else:   # mix: DVE is ~2x gpsimd on f32
                                    eng = (nc.gpsimd if kb % 3 == 2
                                           else nc.vector)
                                if kb == 0:
                                    eng.tensor_copy(st["acc"][:], ph)
                                else:
                                    eng.tensor_add(st["acc"][:],
                                                   st["acc"][:], ph)
                        if kb0 == nkb - gw:         # group done -> finalize
                            del state[key]
                            return lambda: finalize(qt, h, st)
                        return None

                    def finalize(qt, h, st):
                        if qt not in oq_sb:
                            oq_sb[qt] = work.tile([128, HPC, 512], bf,
                                                  tag="oqs", bufs=2,
                                                  name="oq_sb")
                        if ROWSUM == "pe" and RS_BCAST:
                            rb1_sb = work.tile([1, 512], f32,
                                               tag="rs1_sb", bufs=2,
                                               name="rb1_sb")
                            nc.vector.reciprocal(rb1_sb[:], st["rs1"][:])
                            rb_sb = work.tile([128, 512], f32,
                                              tag="rb_sb", bufs=2)
                            nc.gpsimd.partition_broadcast(rb_sb[:],
                                                          rb1_sb[:])
                            nc.vector.tensor_mul(oq_sb[qt][:, h, :],
                                                 st["out"][:], rb_sb[:])
                            if h == HPC - 1:
                                ship(qt)
                            return
                        if ROWSUM == "pe" and PE_LA3:
                            # borrow a scores slot (same shape) so no
                            # dedicated rs bank is needed and LA stays 3
                            rs_ps = ps2.tile([128, SPS], f32, tag="s",
                                             bufs=LA + 1, name="rs_ps")
                        else:
                            rs_ps = ps2.tile([128, 512], f32, tag="rs",
                                             bufs=1 if (LA_DEEP or
                                                        ROWSUM == "pe")
                                             else 2,
                                             name="rs_ps")
                        if ROWSUM == "pe":
                            rs1_sb = work.tile([1, 512], f32,
                                               tag="rs1_sb", bufs=2)
                            nc.scalar.copy(rs1_sb[:], st["rs1"][:])
                            nc.tensor.matmul(rs_ps[:], ones_1x[:],
                                             rs1_sb[:],
                                             start=True, stop=True)
                        else:
                            nc.tensor.matmul(rs_ps[:], ones128[:],
                                             st["acc"][:],
                                             start=True, stop=True)
                        rb_sb = work.tile([128, 512], f32, tag="rb_sb",
                                          bufs=2)
                        nc.vector.reciprocal(rb_sb[:], rs_ps[:])
                        nc.vector.tensor_mul(oq_sb[qt][:, h, :],
                                             st["out"][:], rb_sb[:])
                        if h == HPC - 1:
                            ship(qt)

                    from collections import deque
                    pending = deque()
                    fin = [None]

                    def run_proc(args):
                        f = process(*args)
                        if not FIN_DEFER:
                            if f is not None:
                                f()
                            return
                        if fin[0] is not None:
                            fin[0]()
                        fin[0] = f

                    for t in tasks:
                        pending.append((t, issue_scores(t)))
                        if len(pending) > LA:
                            run_proc(pending.popleft())
                    while pending:
                        run_proc(pending.popleft())
                    if fin[0] is not None:
                        fin[0]()

            def phase2(do_cc=True, par=0):
                attn_tasks(list(range(SC)), par=par)
                if do_cc:
                    a2a_issue(par)

            def phase12(do_cc=True, par=0):
                for sc_i in range(SC):
                    proj_chunk(sc_i)
                    attn_tasks([sc_i], par=par)
                if do_cc:
                    a2a_issue(par)

            # ====== Phase 3: (seq-group, feature-half)-sharded out-proj ==
            def phase3_copy(par=0):
                # land gathered activations in SBUF: all 32 heads for my
                # 512-seq group
                # g_sb copies wait on the collective — issue them on the
                # scalar queue so wop weight prefetches (sync queue) are
                # not head-of-line blocked behind them during the A2A
                for c in range(N_CORES):
                    nc.scalar.dma_start(
                        g_sb[:, bass.ds(c * HPC, HPC), :],
                        a2a_out[par][bass.ds(c * 128, 128), :, :])

            def phase3(par=0, skip_copy=False):
                if not skip_copy:
                    phase3_copy(par)
                with tc.tile_pool(name="ps3", bufs=2, space="PSUM") as ps3:
                    for ft in range(FT):
                        wop = gpool.tile([128, NH, 128], bf, tag="wop",
                                         bufs=WOP_BUFS)
                        nc.sync.dma_start(wop[:], wo[:, ft, :, :])
                        out_ps = ps3.tile([128, 512], f32, tag="op",
                                          name="out_ps")
                        for h in range(NH):
                            nc.tensor.matmul(
                                out_ps[:], wop[:, h, :], g_sb[:, h, :],
                                start=h == 0, stop=h == NH - 1)
                        if Y_DIRECT:
                            nc.scalar.dma_start(y[bass.ts(ft, 128), :],
                                                out_ps[:])
                            continue
                        y_sb = work.tile([128, 512], f32, tag="y_sb",
                                         bufs=2)
                        if ft % 2 == 0:
                            nc.vector.tensor_copy(y_sb[:], out_ps[:])
                        else:
                            nc.scalar.copy(y_sb[:], out_ps[:])
                        # y on the gpsimd queue: keeps the sync queue
                        # free so the next rep's xt prefetch isn't blocked
                        # behind wop slot-waits
                        nc.gpsimd.dma_start(y[bass.ts(ft, 128), :], y_sb[:])

            phase_fns = {1: phase1, 2: phase2, 3: phase3}

            if not timing and reps_phases is not None:
                # static-unrolled per-phase timing builds
                ph = set(reps_phases)
                if 1 not in ph:
                    for t in [k_sb, v_sb] + q_sb:
                        nc.any.memset(t[:], 0.0)
                if 2 not in ph and 3 in ph:
                    z_sb = work.tile([128, HPC, 512], bf, tag="oqs",
                                     bufs=2, name="z_sb")
                    nc.any.memset(z_sb[:], 0.0)
                    for c in range(N_CORES):
                        nc.sync.dma_start(
                            a2a_out[0][bass.ds(c * 128, 128), :, :],
                            z_sb[:])
                for _rep in range(reps):
                    if 1 in ph and 2 in ph and FUSE12:
                        phase12(do_cc=not skip_cc)
                    else:
                        if 1 in ph:
                            phase1()
                        if 2 in ph:
                            phase2(do_cc=not skip_cc)
                    if 3 in ph:
                        phase3()
            elif not timing:
                # PIPE: emit phase3 of rep r after phase1/2 of rep r+1 so
                # the AllToAll flies behind the next rep's projection and
                # attention compute (parity-double-buffered a2a tiles).
                # reps=1 emission is identical either way.
                if PIPE:
                    for _rep in range(reps):
                        par = _rep % 2
                        if FUSE12:
                            phase12(do_cc=not skip_cc, par=par)
                        else:
                            phase1()
                            phase2(do_cc=not skip_cc, par=par)
                        if _rep > 0:
                            phase3(par=1 - par)
                    phase3(par=(reps - 1) % 2)
                else:
                    for _rep in range(reps):
                        if FUSE12:
                            phase12(do_cc=not skip_cc)
                        else:
                            phase1()
                            phase2(do_cc=not skip_cc)
                        phase3()
            else:
                # Collectives cannot sit inside a For_i loop: phase2 runs
                # with do_cc=False in the loop; the A2A is issued once
                # after it. Skipped producer phases get cheap inits so
                # consumers' tiles exist (values irrelevant for timing).
                if phases_only:
                    if 1 not in phases:
                        for t in [k_sb, v_sb] + q_sb:
                            nc.any.memset(t[:], 0.0)
                    if 2 not in phases and 3 in phases:
                        z_sb = work.tile([128, HPC, 512], bf, tag="oqs",
                                         bufs=2, name="z_sb")
                        nc.any.memset(z_sb[:], 0.0)
                        for c in range(N_CORES):
                            nc.sync.dma_start(
                                a2a_out[0][bass.ds(c * 128, 128), :, :],
                                z_sb[:])
                loop_body = []
                for p in phases:
                    if p == 2:
                        loop_body.append(lambda: phase2(do_cc=False))
                    else:
                        loop_body.append(phase_fns[p])
                for p in (1, 2, 3):
                    if (p not in phases and p < min(phases)
                            and not phases_only):
                        phase_fns[p]()
                if timing == 1:
                    for fn in loop_body:
                        fn()
                else:
                    with tc.For_i(0, int(timing), 1):
                        for fn in loop_body:
                            fn()
                if 2 in phases:
                    a2a_issue()
                for p in (1, 2, 3):
                    if p not in phases and p > max(phases) and not phases_only:
                        phase_fns[p]()

    nc.compile()
    return nc


class BassExec:
    """Build-once, run-many SPMD executor over the axon PJRT path.

    Modeled on concourse.bass2jax.run_bass_via_pjrt, but keeps the jitted
    callable so repeated executions skip re-tracing/re-compiling.
    """

    def __init__(self, nc, n_cores):
        import jax
        from jax.sharding import Mesh, PartitionSpec, NamedSharding
        from jax.experimental.shard_map import shard_map
        from concourse import bass2jax
        from concourse.bass2jax import _bass_exec_p, partition_id_tensor

        bass2jax.install_neuronx_cc_hook()
        self.jax = jax
        self.nc = nc
        self.n_cores = n_cores
        partition_name = (nc.partition_id_tensor.name
                          if nc.partition_id_tensor else None)
        in_names, out_names, out_avals, zero_outs = [], [], [], []
        for alloc in nc.m.functions[0].allocations:
            if not isinstance(alloc, mybir.MemoryLocationSet):
                continue
            name = alloc.memorylocations[0].name
            if alloc.kind == "ExternalInput":
                if name != partition_name:
                    in_names.append(name)
            elif alloc.kind == "ExternalOutput":
                out_names.append(name)
                shape = tuple(alloc.tensor_shape)
                dtype = mybir.dt.np(alloc.dtype)
                out_avals.append(jax.core.ShapedArray(shape, dtype))
                zero_outs.append(np.zeros(shape, dtype))
        self.in_names, self.out_names = in_names, out_names
        self.out_avals, self.zero_outs = out_avals, zero_outs
        n_params = len(in_names)
        n_outs = len(out_avals)
        all_in_names = list(in_names) + list(out_names)
        if partition_name is not None:
            all_in_names.append(partition_name)

        def _body(*args):
            operands = list(args)
            if partition_name is not None:
                operands.append(partition_id_tensor())
            outs = _bass_exec_p.bind(
                *operands,
                out_avals=tuple(out_avals),
                in_names=tuple(all_in_names),
                out_names=tuple(out_names),
                lowering_input_output_aliases=(),
                sim_require_finite=True,
                sim_require_nnan=True,
                nc=nc,
            )
            return tuple(outs)

        devices = jax.devices()[:n_cores]
        self.mesh = Mesh(np.asarray(devices), ("core",))
        in_specs = (PartitionSpec("core"),) * (n_params + n_outs)
        out_specs = (PartitionSpec("core"),) * n_outs
        donate = tuple(range(n_params, n_params + n_outs))
        self.sharded = jax.jit(
            shard_map(_body, mesh=self.mesh, in_specs=in_specs,
                      out_specs=out_specs, check_rep=False),
            donate_argnums=donate, keep_unused=True,
        )
        self.sharding = NamedSharding(self.mesh, PartitionSpec("core"))

    def put_inputs(self, in_maps):
        concat = [np.concatenate([np.asarray(in_maps[c][n])
                                  for c in range(self.n_cores)], axis=0)
                  for n in self.in_names]
        return [self.jax.device_put(a, self.sharding) for a in concat]

    def zeros_dev(self):
        return [self.jax.device_put(
            np.zeros((self.n_cores * z.shape[0], *z.shape[1:]), z.dtype),
            self.sharding) for z in self.zero_outs]

    def run(self, ins_dev):
        outs = self.sharded(*ins_dev, *self.zeros_dev())
        self.jax.block_until_ready(outs)
        return outs

    def results(self, outs):
        return [{name: np.asarray(outs[i]).reshape(
                    self.n_cores, *self.out_avals[i].shape)[c]
                 for i, name in enumerate(self.out_names)}
                for c in range(self.n_cores)]


_CACHE = {}


def _get_exec():
    if "exec" not in _CACHE:
        _CACHE["exec"] = BassExec(build_nc(), N_CORES)
    return _CACHE["exec"]


def make_in_maps(hidden_states, position_ids, Wq, Wk, Wv, Wo):
    X = np.asarray(hidden_states)[0]          # [S, H] f32
    pos = np.asarray(position_ids)[0]                      # [S]
    inv = 1.0 / (ROPE_THETA ** (np.arange(0, D, 2, dtype=np.float32) / D))
    inv_full = np.concatenate([inv, inv]).astype(np.float32)   # [128]
    # fp32 product (matches reference's fp32 freqs), then exact range
    # reduction to [-pi, pi) where the ACT Sin unit is accurate
    prod = (pos[None, :].astype(np.float32)
            * inv_full[:, None]).astype(np.float64)
    tp = 2 * np.pi
    fsin = (np.mod(prod + np.pi, tp) - np.pi).astype(np.float32)
    fcos = (np.mod(prod + np.pi / 2 + np.pi, tp) - np.pi).astype(np.float32)

    t = np.arange(1024)[None, :]
    k = np.arange(128)[:, None]
    msk = (t >= k + 512).astype(ml_dtypes.bfloat16)        # [128, 1024]

    xt = np.ascontiguousarray(
        X.reshape(SC, 512, HC, 128).transpose(3, 0, 2, 1)
    ).astype(ml_dtypes.bfloat16)                       # [128, SC, HC, 512]

    in_maps = []
    for c in range(N_CORES):
        wq_c = np.asarray(Wq)[:, c * QO:(c + 1) * QO]       # [H, 512]
        wk_c = np.asarray(Wk)[:, c * D:(c + 1) * D]         # [H, 128]
        wv_c = np.asarray(Wv)[:, c * D:(c + 1) * D]
        fh = c & 1
        wo_c = np.asarray(Wo)[:, fh * FH:(fh + 1) * FH]     # [H, 2048]
        in_maps.append({
            "xt": xt,
            "wq": np.ascontiguousarray(
                wq_c.reshape(HC, 128, QO).transpose(1, 0, 2)
            ).astype(ml_dtypes.bfloat16),
            "wk": np.ascontiguousarray(
                wk_c.reshape(HC, 128, D).transpose(1, 0, 2)
            ).astype(ml_dtypes.bfloat16),
            "wv": np.ascontiguousarray(
                wv_c.reshape(HC, 128, D).transpose(1, 0, 2)
            ).astype(ml_dtypes.bfloat16),
            "wo": np.ascontiguousarray(
                wo_c.reshape(NH, 128, FT, 128).transpose(1, 2, 0, 3)
            ).astype(ml_dtypes.bfloat16),              # [128, FT, NH, 128]
            "fsin": fsin,
            "fcos": fcos,
            "msk": np.ascontiguousarray(msk),
        })
    return in_maps


def assemble_output(results):
    # results[c]["y"]: [2048, 512] = Y^T rows fh*2048.., cols sg*512..
    # with sg = c>>1, fh = c&1
    final_t = np.empty((H, S), np.float32)
    for c in range(N_CORES):
        sg, fh = c >> 1, c & 1
        final_t[fh * FH:(fh + 1) * FH, sg * 512:(sg + 1) * 512] = \
            results[c]["y"]
    return np.ascontiguousarray(final_t.T)[None].astype(np.float32)


def kernel(hidden_states, position_ids, Wq, Wk, Wv, Wo):
    ex = _get_exec()
    in_maps = make_in_maps(hidden_states, position_ids, Wq, Wk, Wv, Wo)
    outs = ex.run(ex.put_inputs(in_maps))
    return assemble_output(ex.results(outs))


if __name__ == "__main__":
    rng = np.random.default_rng(0)
    hs = rng.standard_normal((1, S, H)).astype(np.float32)
    pid = np.broadcast_to(np.arange(S, dtype=np.int32), (1, S))
    Wq_ = (rng.standard_normal((H, NH * D)) * 0.02).astype(np.float32)
    Wk_ = (rng.standard_normal((H, NKV * D)) * 0.02).astype(np.float32)
    Wv_ = (rng.standard_normal((H, NKV * D)) * 0.02).astype(np.float32)
    Wo_ = (rng.standard_normal((NH * D, H)) * 0.02).astype(np.float32)
    out = kernel(hs, pid, Wq_, Wk_, Wv_, Wo_)
    print("out", out.shape, out.dtype, out[0, :2, :4])
